# revision 37
# baseline (speedup 1.0000x reference)
"""Trainium2 Bass kernel for single-query attention pooling.

Problem (hardcoded): queries [32,1,128] f32, values [32,8192,128] f32
  scores  = einsum("bqd,bnd->bn", q, v)          # [32, 8192]
  attn    = softmax(scores, axis=-1)             # [32, 8192]
  context = einsum("bn,bnd->bd", attn, v)        # [32, 128]
returns (attn, context) as float32.

Strategy: data-parallel over batch across 8 NeuronCores (4/core).
Host pre-casts V to fp16 and sends both a natural and a (column-permuted)
transposed copy — total HBM read per core = 16 MiB fp16, the same bytes
as one f32 copy. On-device per batch:
  - scores: 64 matmuls lhsT = contiguous 128-col slice of V^T, rhs=q^T
    -> PSUM scores [128, 64] fp32 (column j=8g+r holds n=1024g+8p+r).
  - softmax: DVE row-max, GPSIMD partition_all_reduce(max), DVE negate,
    ACT exp -> e fp16 + row sums, GPSIMD partition_all_reduce(add),
    DVE reciprocal  (no TensorE in the chain).
  - context: 64 accumulating matmuls lhsT=e column, rhs=V natural tile.
  - attn = e*(1/T), PE transpose, store; host undoes the n-interleave.
The batch loop is software-pipelined: scores(b+1) is emitted before
ctx(b) so the TensorE never waits on the softmax chain.
"""

import numpy as np

B, N, D = 32, 8192, 128
NCORES = 8
BPC = B // NCORES          # batches per core
P = 128
R = 64                     # consecutive rows per partition in V natural
NGROUPS = N // (P * R)     # 8 groups per batch
TILES = N // P             # 64 score/ctx tiles per batch
NCHUNK = 1                 # V^T load chunks per batch
CW = N // NCHUNK
TPC = TILES // NCHUNK      # score tiles per V^T chunk

_cache = {}


def build_bass():
    if "nc" in _cache:
        return _cache["nc"]
    import concourse.tile as tile
    from concourse import bacc, bass_isa, mybir
    from concourse.masks import make_identity

    f32 = mybir.dt.float32
    f16 = mybir.dt.float16
    X = mybir.AxisListType.X
    Exp = mybir.ActivationFunctionType.Exp

    # Lighter kernel-exit: the second all-engine barrier after the sem
    # clear only needs sem visibility (engines were already drained by the
    # first barrier); drop its drains to shave the exit butterfly.
    if not getattr(tile.TileContext, "_light_exit_patched", False):
        _ScopedClock = tile.ScopedClock

        def _light_drain_and_barrier(self, tick_clock, wait_clock):
            drain_inst = self.nc.sync.drain()
            wait_clock.add_sem_waits(
                drain_inst.ins, _ScopedClock({None: tick_clock.global_clock})
            )
            self.nc.all_engine_barrier()
            assert self.sems is not None
            popped = self.nc._tile_sem_poison_stack.pop()
            assert popped is self._sem_poison
            self.nc.clear_and_free_semaphores(
                list(self.sems.allocated().values())
            )
            self.nc.all_engine_barrier(sem_only=True)

        tile.TileContext._drain_and_barrier = _light_drain_and_barrier
        tile.TileContext._light_exit_patched = True

    nc = bacc.Bacc()
    qT_d = nc.declare_dram_parameter("queries_t", [D, BPC], f16, isOutput=False)
    vh_d = nc.declare_dram_parameter("values_h", [BPC, N, D], f16, isOutput=False)
    vt_d = nc.declare_dram_parameter("values_t", [BPC, D, N], f16, isOutput=False)
    attn_d = nc.declare_dram_parameter("attn", [BPC, N], f16, isOutput=True)
    ctx_d = nc.declare_dram_parameter("context", [BPC, 4, D], f32, isOutput=True)
    rs_d = nc.declare_dram_parameter("rowsums", [P, BPC], f32, isOutput=True)

    with tile.TileContext(nc) as tc:
        with (
            tc.tile_pool(name="singles", bufs=1) as singles,
            tc.tile_pool(name="vtx", bufs=3) as vtp,
            tc.tile_pool(name="vg", bufs=3) as vgp,
            tc.tile_pool(name="small", bufs=3) as smallp,
            tc.tile_pool(name="ps_sc", bufs=2, space="PSUM") as ps_sc,
            tc.tile_pool(name="ps_ctx", bufs=2, space="PSUM") as ps_ctx,
            tc.tile_pool(name="ps_misc", bufs=2, space="PSUM") as ps_misc,
        ):
            st = [dict() for _ in range(BPC)]

            def emit_loads(b):
                s = st[b]
                s["vt"] = []
                for c in range(NCHUNK):
                    vtc = vtp.tile([P, CW], f16, tag="vtc")
                    nc.sync.dma_start(
                        out=vtc, in_=vt_d[b][:, c * CW : (c + 1) * CW]
                    )
                    s["vt"].append(vtc)
                vg = vgp.tile([P, NGROUPS, R, D], f16, tag="vg")
                vh_v = vh_d[b].rearrange("(g p r) d -> p g r d", p=P, r=R)
                rh = R // 2
                nc.scalar.dma_start(out=vg[:, :, :rh], in_=vh_v[:, :, :rh])
                nc.scalar.dma_start(out=vg[:, :, rh:], in_=vh_v[:, :, rh:])
                s["vg"] = vg

            def emit_scores(b):
                s = st[b]
                sc_ps = ps_sc.tile([P, TILES], f32, tag="sc")
                for j in range(TILES):
                    c, t = divmod(j, TPC)
                    nc.tensor.matmul(
                        sc_ps[:, j : j + 1],
                        lhsT=s["vt"][c][:, t * P : (t + 1) * P],
                        rhs=qT[:, b : b + 1],
                        start=True,
                        stop=True,
                    )
                s["sc_ps"] = sc_ps

            def emit_chain(b):
                s = st[b]
                sc_ps = s["sc_ps"]
                m_sb = smallp.tile([P, 1], f32, tag="m")
                nc.vector.tensor_reduce(
                    out=m_sb, in_=sc_ps, axis=X, op=mybir.AluOpType.max
                )
                mbc = smallp.tile([P, 1], f32, tag="mbc")
                nc.gpsimd.partition_all_reduce(
                    mbc, m_sb, channels=P, reduce_op=bass_isa.ReduceOp.max
                )
                nm_sb = smallp.tile([P, 1], f32, tag="nm")
                nc.vector.tensor_scalar_mul(nm_sb, mbc, -1.0)
                e16 = smallp.tile([P, TILES], f16, tag="e16")
                rowsum = smallp.tile([P, 1], f32, tag="rowsum")
                nc.scalar.activation(
                    out=e16,
                    in_=sc_ps,
                    func=Exp,
                    bias=nm_sb,
                    scale=1.0,
                    accum_out=rowsum,
                )
                s["e16"] = e16
                s["rowsum"] = rowsum

            def emit_ctx_attn(b):
                s = st[b]
                # attn (= unnormalized e) transpose + store first: it only
                # needs e16, so the store overlaps the ctx matmul block.
                attnT_ps = ps_misc.tile([TILES, P], f16, tag="misc")
                nc.tensor.transpose(attnT_ps, s["e16"], ident_h)
                attnT_sb = smallp.tile([TILES, P], f16, tag="attnT")
                if b == BPC - 1:
                    nc.vector.tensor_copy(attnT_sb, attnT_ps)
                else:
                    nc.scalar.copy(attnT_sb, attnT_ps)
                # last batch: loads are done, use the lower-latency HWDGE ring
                st_eng = nc.sync if b == BPC - 1 else nc.gpsimd
                st_eng.dma_start(
                    out=attn_d[b].rearrange("(j p) -> j p", p=P), in_=attnT_sb
                )
                # 4x column-packed context matmuls: col-group c accumulates
                # tiles j%4==c into a partial row at partition 32c.
                QUAD = 4
                NQ = TILES // QUAD
                ctx_ps = ps_ctx.tile([97, D], f32, tag="ctx")
                # zero the gap partitions so one [97, D] evac copy reads no
                # garbage (the matmuls only write rows 0/32/64/96)
                nc.vector.memset(ctx_ps, 0.0)
                for jq in range(NQ):
                    for c in range(QUAD):
                        j = jq * QUAD + c
                        g, r = divmod(j, R)
                        nc.tensor.matmul(
                            ctx_ps[32 * c : 32 * c + 1, :],
                            lhsT=s["e16"][:, j : j + 1],
                            rhs=s["vg"][:, g, r, :],
                            start=(jq == 0),
                            stop=(jq == NQ - 1),
                            tile_position=(0, 32 * c),
                        )
                ctx4_sb = smallp.tile([97, D], f32, tag="ctxrow")
                nc.vector.tensor_copy(ctx4_sb, ctx_ps)
                st_eng.dma_start(
                    out=ctx_d[b], in_=ctx4_sb[::32, :]
                )
                nc.vector.tensor_copy(rs_sb[:, b : b + 1], s["rowsum"])

            qT = singles.tile([P, BPC], f16)
            nc.sync.dma_start(out=qT, in_=qT_d[:])
            rs_sb = singles.tile([P, BPC], f32)
            emit_loads(0)
            ident_h = singles.tile([P, P], f16)
            make_identity(nc, ident_h)
            if BPC > 1:
                emit_loads(1)
            emit_scores(0)
            for b in range(BPC):
                if b + 2 < BPC:
                    emit_loads(b + 2)
                emit_chain(b)
                if b + 1 < BPC:
                    emit_scores(b + 1)
                emit_ctx_attn(b)
            nc.sync.dma_start(out=rs_d[:], in_=rs_sb)

    nc.finalize()
    _cache["nc"] = nc
    return nc


def prepare_inputs(queries, values):
    """Host-side prep: fp16 cast + transposed V copy, sliced per core."""
    q16 = np.asarray(queries, np.float16).reshape(B, D)
    v16 = np.asarray(values, np.float16)
    qt = np.ascontiguousarray(q16.T)                      # [D, B]
    # V^T with columns permuted so that scores tile j = 8g+r reads a
    # contiguous 128-column block: position 128*(8g+r)+p <-> n=1024g+8p+r
    vt = np.ascontiguousarray(
        v16.reshape(B, NGROUPS, P, R, D)
        .transpose(0, 4, 1, 3, 2)
        .reshape(B, D, N)
    )
    in_maps = []
    for i in range(NCORES):
        s = slice(i * BPC, (i + 1) * BPC)
        in_maps.append(
            {
                "queries_t": np.ascontiguousarray(qt[:, s]),
                "values_h": np.ascontiguousarray(v16[s]),
                "values_t": np.ascontiguousarray(vt[s]),
            }
        )
    return in_maps


def finish_outputs(results):
    """Gather per-core outputs; undo the attn n-interleave on host."""
    e_k = np.concatenate(
        [np.asarray(results[i]["attn"]) for i in range(NCORES)], 0
    )
    ctx4 = np.concatenate(
        [np.asarray(results[i]["context"]) for i in range(NCORES)], 0
    ).sum(axis=1, dtype=np.float32)
    T = np.concatenate(
        [np.asarray(results[i]["rowsums"]).sum(axis=0, dtype=np.float32)
         for i in range(NCORES)]
    )  # [B]
    # kernel order: m = 128*(8g + r) + p  <->  true n = 1024g + 8p + r
    attn = (
        e_k.astype(np.float32)
        .reshape(B, NGROUPS, R, P)
        .transpose(0, 1, 3, 2)
        .reshape(B, N)
    ) / T[:, None]
    ctx = ctx4 / T[:, None]
    return np.ascontiguousarray(attn, np.float32), np.ascontiguousarray(
        ctx, np.float32
    )


def _ensure_axon_hooks():
    """This image's antenv lacks axon_hooks; register a ctypes-based NTFF
    profile hook so run_bass_kernel_spmd(trace=True) degrades gracefully
    instead of crashing on the import (mirrors trn_boot's hook)."""
    import sys

    if "antenv.axon_hooks" in sys.modules:
        return
    try:
        import antenv.axon_hooks  # noqa: F401

        return
    except ImportError:
        pass
    import contextlib
    import ctypes
    import types

    try:
        lib = ctypes.CDLL("/opt/axon/libaxon_pjrt.so")
        lib.axon_start_nrt_profile.argtypes = [
            ctypes.POINTER(ctypes.c_int64),
            ctypes.c_size_t,
        ]
        lib.axon_start_nrt_profile.restype = ctypes.c_int64
        lib.axon_stop_nrt_profile.argtypes = [ctypes.c_char_p]
        lib.axon_stop_nrt_profile.restype = ctypes.c_int64
    except (OSError, AttributeError):
        lib = None

    @contextlib.contextmanager
    def _hook(output_dir, device_ids):
        if lib is None:
            yield
            return
        import jax

        jax.devices()
        if device_ids:
            ids = (ctypes.c_int64 * len(device_ids))(*device_ids)
            lib.axon_start_nrt_profile(ids, len(device_ids))
        else:
            lib.axon_start_nrt_profile(None, 0)
        try:
            yield
        finally:
            lib.axon_stop_nrt_profile(str(output_dir).encode())

    mod = types.ModuleType("antenv.axon_hooks")
    mod.get_axon_ntff_profile_hook = lambda: (_hook if lib is not None else None)
    mod.set_axon_ntff_profile_hook = lambda h: None
    sys.modules["antenv.axon_hooks"] = mod


def kernel(queries, values):
    _ensure_axon_hooks()
    from concourse.bass_utils import run_bass_kernel_spmd

    nc = build_bass()
    in_maps = prepare_inputs(queries, values)
    res = run_bass_kernel_spmd(nc, in_maps, list(range(NCORES)))
    return finish_outputs(res.results)


# revision 39
# speedup vs baseline: 1.0535x; 1.0535x over previous
"""Trainium2 Bass kernel for single-query attention pooling.

Problem (hardcoded): queries [32,1,128] f32, values [32,8192,128] f32
  scores  = einsum("bqd,bnd->bn", q, v)          # [32, 8192]
  attn    = softmax(scores, axis=-1)             # [32, 8192]
  context = einsum("bn,bnd->bd", attn, v)        # [32, 128]
returns (attn, context) as float32.

Strategy: data-parallel over batch across 8 NeuronCores (4/core).
Host pre-casts V to fp16 and sends both a natural and a (column-permuted)
transposed copy — total HBM read per core = 16 MiB fp16, the same bytes
as one f32 copy. On-device per batch:
  - scores: 64 matmuls lhsT = contiguous 128-col slice of V^T, rhs=q^T
    -> PSUM scores [128, 64] fp32 (column j=8g+r holds n=1024g+8p+r).
  - softmax: DVE row-max, GPSIMD partition_all_reduce(max), DVE negate,
    ACT exp -> e fp16 + row sums, GPSIMD partition_all_reduce(add),
    DVE reciprocal  (no TensorE in the chain).
  - context: 64 accumulating matmuls lhsT=e column, rhs=V natural tile.
  - attn = e*(1/T), PE transpose, store; host undoes the n-interleave.
The batch loop is software-pipelined: scores(b+1) is emitted before
ctx(b) so the TensorE never waits on the softmax chain.
"""

import numpy as np

B, N, D = 32, 8192, 128
NCORES = 8
BPC = B // NCORES          # batches per core
P = 128
R = 64                     # consecutive rows per partition in V natural
NGROUPS = N // (P * R)     # 8 groups per batch
TILES = N // P             # 64 score/ctx tiles per batch
NCHUNK = 1                 # V^T load chunks per batch
CW = N // NCHUNK
TPC = TILES // NCHUNK      # score tiles per V^T chunk

_cache = {}


def build_bass():
    if "nc" in _cache:
        return _cache["nc"]
    import concourse.tile as tile
    from concourse import bacc, bass_isa, mybir
    from concourse.masks import make_identity

    f32 = mybir.dt.float32
    f16 = mybir.dt.float16
    X = mybir.AxisListType.X
    Exp = mybir.ActivationFunctionType.Exp

    # Lighter kernel-exit: the second all-engine barrier after the sem
    # clear only needs sem visibility (engines were already drained by the
    # first barrier); drop its drains to shave the exit butterfly.
    if not getattr(tile.TileContext, "_light_exit_patched", False):
        _ScopedClock = tile.ScopedClock

        def _light_drain_and_barrier(self, tick_clock, wait_clock):
            drain_inst = self.nc.sync.drain()
            wait_clock.add_sem_waits(
                drain_inst.ins, _ScopedClock({None: tick_clock.global_clock})
            )
            self.nc.all_engine_barrier()
            assert self.sems is not None
            popped = self.nc._tile_sem_poison_stack.pop()
            assert popped is self._sem_poison
            self.nc.clear_and_free_semaphores(
                list(self.sems.allocated().values())
            )
            self.nc.all_engine_barrier(sem_only=True)

        tile.TileContext._drain_and_barrier = _light_drain_and_barrier
        tile.TileContext._light_exit_patched = True

    nc = bacc.Bacc()
    qT_d = nc.declare_dram_parameter("queries_t", [D, BPC], f16, isOutput=False)
    vh_d = nc.declare_dram_parameter("values_h", [BPC, N, D], f16, isOutput=False)
    vt_d = nc.declare_dram_parameter("values_t", [BPC, D, N], f16, isOutput=False)
    attn_d = nc.declare_dram_parameter("attn", [BPC, N], f16, isOutput=True)
    ctx_d = nc.declare_dram_parameter("context", [BPC, 4, D], f32, isOutput=True)
    rs_d = nc.declare_dram_parameter("rowsums", [P, BPC], f32, isOutput=True)

    with tile.TileContext(nc) as tc:
        with (
            tc.tile_pool(name="singles", bufs=1) as singles,
            tc.tile_pool(name="vtx", bufs=3) as vtp,
            tc.tile_pool(name="vg", bufs=3) as vgp,
            tc.tile_pool(name="small", bufs=3) as smallp,
            tc.tile_pool(name="ps_sc", bufs=2, space="PSUM") as ps_sc,
            tc.tile_pool(name="ps_ctx", bufs=2, space="PSUM") as ps_ctx,
            tc.tile_pool(name="ps_misc", bufs=2, space="PSUM") as ps_misc,
        ):
            st = [dict() for _ in range(BPC)]

            def emit_loads(b):
                s = st[b]
                s["vt"] = []
                for c in range(NCHUNK):
                    vtc = vtp.tile([P, CW], f16, tag="vtc")
                    nc.sync.dma_start(
                        out=vtc, in_=vt_d[b][:, c * CW : (c + 1) * CW]
                    )
                    s["vt"].append(vtc)
                vg = vgp.tile([P, NGROUPS, R, D], f16, tag="vg")
                vh_v = vh_d[b].rearrange("(g p r) d -> p g r d", p=P, r=R)
                rh = R // 2
                nc.scalar.dma_start(out=vg[:, :, :rh], in_=vh_v[:, :, :rh])
                nc.scalar.dma_start(out=vg[:, :, rh:], in_=vh_v[:, :, rh:])
                s["vg"] = vg

            def emit_scores(b):
                s = st[b]
                sc_ps = ps_sc.tile([P, TILES], f32, tag="sc")
                for j in range(TILES):
                    c, t = divmod(j, TPC)
                    nc.tensor.matmul(
                        sc_ps[:, j : j + 1],
                        lhsT=s["vt"][c][:, t * P : (t + 1) * P],
                        rhs=qT[:, b : b + 1],
                        start=True,
                        stop=True,
                    )
                s["sc_ps"] = sc_ps

            def emit_chain(b):
                s = st[b]
                sc_ps = s["sc_ps"]
                m_sb = smallp.tile([P, 1], f32, tag="m")
                nc.vector.tensor_reduce(
                    out=m_sb, in_=sc_ps, axis=X, op=mybir.AluOpType.max
                )
                mbc = smallp.tile([P, 1], f32, tag="mbc")
                nc.gpsimd.partition_all_reduce(
                    mbc, m_sb, channels=P, reduce_op=bass_isa.ReduceOp.max
                )
                nm_sb = smallp.tile([P, 1], f32, tag="nm")
                nc.vector.tensor_scalar_mul(nm_sb, mbc, -1.0)
                e16 = smallp.tile([P, TILES], f16, tag="e16")
                rowsum = smallp.tile([P, 1], f32, tag="rowsum")
                nc.scalar.activation(
                    out=e16,
                    in_=sc_ps,
                    func=Exp,
                    bias=nm_sb,
                    scale=1.0,
                    accum_out=rowsum,
                )
                s["e16"] = e16
                s["rowsum"] = rowsum

            def emit_ctx_attn(b):
                s = st[b]
                # attn (= unnormalized e) transpose + store first: it only
                # needs e16, so the store overlaps the ctx matmul block.
                attnT_ps = ps_misc.tile([TILES, P], f16, tag="misc")
                nc.tensor.transpose(attnT_ps, s["e16"], ident_h)
                attnT_sb = smallp.tile([TILES, P], f16, tag="attnT")
                if b == BPC - 1:
                    nc.vector.tensor_copy(attnT_sb, attnT_ps)
                else:
                    nc.scalar.copy(attnT_sb, attnT_ps)
                # last batch: loads are done, use the lower-latency HWDGE ring
                st_eng = nc.sync if b == BPC - 1 else nc.gpsimd
                st_eng.dma_start(
                    out=attn_d[b].rearrange("(j p) -> j p", p=P), in_=attnT_sb
                )
                # 4x column-packed context matmuls: col-group c accumulates
                # tiles j%4==c into a partial row at partition 32c.
                QUAD = 4
                NQ = TILES // QUAD
                ctx_ps = ps_ctx.tile([97, D], f32, tag="ctx")
                # zero the gap partitions so one [97, D] evac copy reads no
                # garbage (the matmuls only write rows 0/32/64/96)
                nc.vector.memset(ctx_ps, 0.0)
                for jq in range(NQ):
                    for c in range(QUAD):
                        j = jq * QUAD + c
                        g, r = divmod(j, R)
                        nc.tensor.matmul(
                            ctx_ps[32 * c : 32 * c + 1, :],
                            lhsT=s["e16"][:, j : j + 1],
                            rhs=s["vg"][:, g, r, :],
                            start=(jq == 0),
                            stop=(jq == NQ - 1),
                            tile_position=(0, 32 * c),
                        )
                ctx4_sb = smallp.tile([97, D], f32, tag="ctxrow")
                nc.vector.tensor_copy(ctx4_sb, ctx_ps)
                st_eng.dma_start(
                    out=ctx_d[b], in_=ctx4_sb[::32, :]
                )
                nc.vector.tensor_copy(rs_sb[:, b : b + 1], s["rowsum"])

            qT = singles.tile([P, BPC], f16)
            nc.sync.dma_start(out=qT, in_=qT_d[:])
            rs_sb = singles.tile([P, BPC], f32)
            emit_loads(0)
            ident_h = singles.tile([P, P], f16)
            make_identity(nc, ident_h)
            if BPC > 1:
                emit_loads(1)
            emit_scores(0)
            for b in range(BPC):
                if b + 2 < BPC:
                    emit_loads(b + 2)
                emit_chain(b)
                if b + 1 < BPC:
                    emit_scores(b + 1)
                emit_ctx_attn(b)
            nc.sync.dma_start(out=rs_d[:], in_=rs_sb)

    nc.finalize()
    _cache["nc"] = nc
    return nc


def prepare_inputs(queries, values):
    """Host-side prep: fp16 cast + transposed V copy, sliced per core."""
    q16 = np.asarray(queries, np.float16).reshape(B, D)
    v16 = np.asarray(values, np.float16)
    qt = np.ascontiguousarray(q16.T)                      # [D, B]
    # V^T with columns permuted so that scores tile j = 8g+r reads a
    # contiguous 128-column block: position 128*(8g+r)+p <-> n=1024g+8p+r
    vt = np.ascontiguousarray(
        v16.reshape(B, NGROUPS, P, R, D)
        .transpose(0, 4, 1, 3, 2)
        .reshape(B, D, N)
    )
    in_maps = []
    for i in range(NCORES):
        s = slice(i * BPC, (i + 1) * BPC)
        in_maps.append(
            {
                "queries_t": np.ascontiguousarray(qt[:, s]),
                "values_h": np.ascontiguousarray(v16[s]),
                "values_t": np.ascontiguousarray(vt[s]),
            }
        )
    return in_maps


def finish_outputs(results):
    """Gather per-core outputs; undo the attn n-interleave on host."""
    e_k = np.concatenate(
        [np.asarray(results[i]["attn"]) for i in range(NCORES)], 0
    )
    ctx4 = np.concatenate(
        [np.asarray(results[i]["context"]) for i in range(NCORES)], 0
    ).sum(axis=1, dtype=np.float32)
    T = np.concatenate(
        [np.asarray(results[i]["rowsums"]).sum(axis=0, dtype=np.float32)
         for i in range(NCORES)]
    )  # [B]
    # kernel order: m = 128*(8g + r) + p  <->  true n = 1024g + 8p + r
    attn = (
        e_k.astype(np.float32)
        .reshape(B, NGROUPS, R, P)
        .transpose(0, 1, 3, 2)
        .reshape(B, N)
    ) / T[:, None]
    ctx = ctx4 / T[:, None]
    return np.ascontiguousarray(attn, np.float32), np.ascontiguousarray(
        ctx, np.float32
    )


def _ensure_axon_hooks():
    """This image's antenv lacks axon_hooks; register a ctypes-based NTFF
    profile hook so run_bass_kernel_spmd(trace=True) degrades gracefully
    instead of crashing on the import (mirrors trn_boot's hook)."""
    import sys

    if "antenv.axon_hooks" in sys.modules:
        return
    try:
        import antenv.axon_hooks  # noqa: F401

        return
    except ImportError:
        pass
    import contextlib
    import ctypes
    import types

    try:
        lib = ctypes.CDLL("/opt/axon/libaxon_pjrt.so")
        lib.axon_start_nrt_profile.argtypes = [
            ctypes.POINTER(ctypes.c_int64),
            ctypes.c_size_t,
        ]
        lib.axon_start_nrt_profile.restype = ctypes.c_int64
        lib.axon_stop_nrt_profile.argtypes = [ctypes.c_char_p]
        lib.axon_stop_nrt_profile.restype = ctypes.c_int64
    except (OSError, AttributeError):
        lib = None

    @contextlib.contextmanager
    def _hook(output_dir, device_ids):
        if lib is None:
            yield
            return
        import jax

        jax.devices()
        if device_ids:
            ids = (ctypes.c_int64 * len(device_ids))(*device_ids)
            lib.axon_start_nrt_profile(ids, len(device_ids))
        else:
            lib.axon_start_nrt_profile(None, 0)
        try:
            yield
        finally:
            lib.axon_stop_nrt_profile(str(output_dir).encode())

    mod = types.ModuleType("antenv.axon_hooks")
    mod.get_axon_ntff_profile_hook = lambda: (_hook if lib is not None else None)
    mod.set_axon_ntff_profile_hook = lambda h: None
    sys.modules["antenv.axon_hooks"] = mod


def kernel(queries, values):
    _ensure_axon_hooks()
    from concourse.bass_utils import run_bass_kernel_spmd

    nc = build_bass()
    in_maps = prepare_inputs(queries, values)
    res = run_bass_kernel_spmd(nc, in_maps, list(range(NCORES)))
    return finish_outputs(res.results)
